# revision 1
# baseline (speedup 1.0000x reference)
"""Trainium2 Bass kernel for decode-style BERT MH self-attention.

Reference computes (B=16, T=8192, C=1024, H=16, D=64):
    x_pe = x + sinusoidal_pe(T, C)
    q  = x_pe[:, :1, :] @ Wq + bq                  (single-query decode)
    kv = x_pe @ Wkv + bkv ; k, v = split
    y  = softmax(q k^T / sqrt(D)) v   -> merge heads -> y @ Wo + bo

Because there is a single query per (b, h), the full K/V projections
(550 GFLOP) collapse algebraically:
    scores[b,h,t] = (Wk_h q_bh) . x_pe[b,t]  + const(b,h)   [const dropped:
                                                             softmax shift-inv]
    y[b,h]        = (attn_b,h . x_pe[b]) @ Wv_h + bv_h      [sum(attn)=1]
so the kernel is one streaming pass over x (~9 GFLOP total), memory-bound.

Sharding: batch B=16 -> 2 per NeuronCore across 8 cores (data parallel,
no collectives). Matmuls run as float32r (TF32-like, full PE rate).
"""
import math
import sys

sys.path.insert(0, "/opt/trn_rl_repo")

import numpy as np

import concourse.bass as bass
import concourse.mybir as mybir
import concourse.tile as tile
from concourse import bacc
from concourse.bass_utils import run_bass_kernel_spmd
from concourse.masks import make_identity

F32 = mybir.dt.float32
F32R = mybir.dt.float32r

B, T, C, H, D = 16, 8192, 1024, 16, 64
NCORES = 8
BL = B // NCORES          # batches per core = 2
TCH = 512                 # t-chunk (streaming granularity)
NCH = T // TCH            # 16 chunks
G = TCH // 128            # 4 sub-tiles of 128 t per chunk
KK = C // 128             # 8 contraction tiles over channels
AFT = mybir.ActivationFunctionType


def build_nc(repeat: int = 1):
    nc = bacc.Bacc("TRN2", target_bir_lowering=False, debug=False,
                   num_devices=NCORES)

    x_d = nc.dram_tensor("x", [BL, T, C], F32, kind="ExternalInput").ap()
    pe_d = nc.dram_tensor("pe", [T, C], F32, kind="ExternalInput").ap()
    x0t_d = nc.dram_tensor("x0T", [C, BL], F32, kind="ExternalInput").ap()
    pe0t_d = nc.dram_tensor("pe0T", [C, BL], F32, kind="ExternalInput").ap()
    wq_d = nc.dram_tensor("Wq", [C, C], F32, kind="ExternalInput").ap()
    wkt_d = nc.dram_tensor("WkT", [C, C], F32, kind="ExternalInput").ap()
    wv_d = nc.dram_tensor("Wv", [C, C], F32, kind="ExternalInput").ap()
    wo_d = nc.dram_tensor("Wo", [C, C], F32, kind="ExternalInput").ap()
    bq_d = nc.dram_tensor("bq2", [BL, C], F32, kind="ExternalInput").ap()
    bv_d = nc.dram_tensor("bv2", [BL, C], F32, kind="ExternalInput").ap()
    bo_d = nc.dram_tensor("bo2", [BL, C], F32, kind="ExternalInput").ap()
    out_d = nc.dram_tensor("out", [BL, C], F32, kind="ExternalOutput").ap()

    with tile.TileContext(nc) as tc:
        with tc.tile_pool(name="const", bufs=1) as cpool:
            ident = cpool.tile([128, 128], F32)
            make_identity(nc, ident[:])
            ident_r = cpool.tile([128, 128], F32R)
            nc.vector.tensor_copy(ident_r[:], ident[:])

            # ---------------- PRE: q and qk^T ----------------
            prew = tc.alloc_tile_pool(name="prew", bufs=1)
            wq_sb = prew.tile([128, KK, C], F32)
            wkt_sb = prew.tile([128, KK, C], F32)
            nc.sync.dma_start(wq_sb[:], wq_d.rearrange("(k p) n -> p k n", p=128))
            nc.sync.dma_start(wkt_sb[:], wkt_d.rearrange("(k p) n -> p k n", p=128))

            x0t_sb = cpool.tile([128, KK, BL], F32)
            pe0t_sb = cpool.tile([128, KK, BL], F32)
            nc.sync.dma_start(x0t_sb[:], x0t_d.rearrange("(k p) b -> p k b", p=128))
            nc.sync.dma_start(pe0t_sb[:], pe0t_d.rearrange("(k p) b -> p k b", p=128))
            xp0 = cpool.tile([128, KK, BL], F32)
            nc.vector.tensor_add(xp0[:], x0t_sb[:], pe0t_sb[:])

            bq_sb = cpool.tile([BL, C], F32)
            nc.sync.dma_start(bq_sb[:], bq_d[:])

            with tc.tile_pool(name="pre_ps", bufs=1, space="PSUM") as pps:
                # q = x_pe0 @ Wq + bq  -> (BL, C)
                q_ps = pps.tile([BL, C], F32)
                for k in range(KK):
                    for nh in range(2):
                        nc.tensor.matmul(
                            q_ps[:, nh * 512:(nh + 1) * 512],
                            xp0[:, k, :],
                            wq_sb[:, k, nh * 512:(nh + 1) * 512],
                            start=(k == 0), stop=(k == KK - 1),
                        )
                q_sb = cpool.tile([BL, C], F32)
                nc.vector.tensor_add(q_sb[:], q_ps[:], bq_sb[:])

                # transpose q -> qT (C, BL) as (128, KK, BL)
                qt_ps = pps.tile([128, 128], F32)
                qt_sb = cpool.tile([128, KK, BL], F32)
                for k in range(KK):
                    nc.tensor.transpose(qt_ps[:, 0:BL], q_sb[:, k * 128:(k + 1) * 128],
                                        ident[0:BL, 0:BL])
                    nc.scalar.activation(qt_sb[:, k, :], qt_ps[:, 0:BL], AFT.Copy)

                # qkT[i, b*H+h] = sum_d WkT[h*D+d, i] * qT[h*D+d, b], scaled
                qk_ps = pps.tile([128, KK, BL * H], F32)
                for h in range(H):
                    pp = (h % 2) * 64
                    kj = h // 2
                    for m in range(KK):
                        nc.tensor.matmul(
                            qk_ps[:, m, h::H],
                            wkt_sb[pp:pp + 64, kj, m * 128:(m + 1) * 128],
                            qt_sb[pp:pp + 64, kj, :],
                            start=True, stop=True,
                        )
                qk_sb = cpool.tile([128, KK, BL * H], F32R)
                # fold in the 1/sqrt(D) attention scale
                nc.scalar.activation(qk_sb[:], qk_ps[:], AFT.Copy,
                                     scale=1.0 / math.sqrt(D))
            prew.release()

            # ---------------- STREAM over t ----------------
            sums = cpool.tile([H, BL, NCH], F32)
            nc.gpsimd.memset(sums[:], 1.0)

            with (
                tc.tile_pool(name="xin", bufs=2) as xin_pool,
                tc.tile_pool(name="pein", bufs=2) as pe_pool,
                tc.tile_pool(name="xp", bufs=1) as xp_pool,
                tc.tile_pool(name="xpt", bufs=1) as xpt_pool,
                tc.tile_pool(name="sc_sb", bufs=2) as sc_pool,
                tc.tile_pool(name="at_sb", bufs=2) as at_pool,
                tc.tile_pool(name="tp_ps", bufs=2, space="PSUM") as tp_ps,
                tc.tile_pool(name="sc_ps", bufs=1, space="PSUM") as sc_ps,
                tc.tile_pool(name="at_ps", bufs=1, space="PSUM") as at_ps,
                tc.tile_pool(name="z_ps", bufs=1, space="PSUM") as z_ps_pool,
            ):
                z_ps = [z_ps_pool.tile([H, C], F32, tag=f"z{b}",
                                       name=f"z_ps{b}")
                        for b in range(BL)]

                def stream_body(_iv=None):
                    for tau in range(NCH):
                        pe_t = pe_pool.tile([128, G, C], F32)
                        nc.sync.dma_start(
                            pe_t[:],
                            pe_d[tau * TCH:(tau + 1) * TCH, :]
                            .rearrange("(g p) i -> p g i", p=128),
                        )
                        for b in range(BL):
                            x_t = xin_pool.tile([128, G, C], F32)
                            nc.sync.dma_start(
                                x_t[:],
                                x_d[b, tau * TCH:(tau + 1) * TCH, :]
                                .rearrange("(g p) i -> p g i", p=128),
                            )
                            xp = xp_pool.tile([128, G, C], F32R, tag=f"xp{b}")
                            nc.vector.tensor_add(xp[:], x_t[:], pe_t[:])

                            # transpose xp -> xpT (i-part, t-free)
                            xpt = xpt_pool.tile([128, KK, TCH], F32R,
                                                tag=f"xpt{b}")
                            for k in range(KK):
                                tp = tp_ps.tile([128, TCH], F32R)
                                for g in range(G):
                                    nc.tensor.transpose(
                                        tp[:, g * 128:(g + 1) * 128],
                                        xp[:, g, k * 128:(k + 1) * 128],
                                        ident_r[:],
                                    )
                                if k % 2 == 0:
                                    nc.scalar.activation(xpt[:, k, :], tp[:],
                                                         AFT.Copy)
                                else:
                                    nc.vector.tensor_copy(xpt[:, k, :], tp[:])

                            # scoresT (h-part, t-free), accumulate over k
                            sc = sc_ps.tile([H, TCH], F32)
                            for k in range(KK):
                                nc.tensor.matmul(
                                    sc[:],
                                    qk_sb[:, k, b * H:(b + 1) * H],
                                    xpt[:, k, :],
                                    start=(k == 0), stop=(k == KK - 1),
                                )
                            # exp (no max subtraction; scores are O(10))
                            at_t = sc_pool.tile([H, TCH], F32, tag=f"att{b}")
                            nc.scalar.activation(
                                at_t[:], sc[:], AFT.Exp,
                                accum_out=sums[:, b, tau:tau + 1],
                            )
                            # transpose attn chunks to (t-part, h-free)
                            attn = at_pool.tile([128, G, H], F32R, tag=f"attn{b}")
                            ap_ps = at_ps.tile([128, G, H], F32)
                            for g in range(G):
                                nc.tensor.transpose(
                                    ap_ps[:, g, :],
                                    at_t[:, g * 128:(g + 1) * 128],
                                    ident[0:H, 0:H],
                                )
                            nc.scalar.activation(attn[:], ap_ps[:], AFT.Copy)
                            # z += attn^T @ xp   (PSUM accumulate across chunks)
                            for g in range(G):
                                for nh in range(2):
                                    nc.tensor.matmul(
                                        z_ps[b][:, nh * 512:(nh + 1) * 512],
                                        attn[:, g, :],
                                        xp[:, g, nh * 512:(nh + 1) * 512],
                                        start=(tau == 0 and g == 0),
                                        stop=(tau == NCH - 1 and g == G - 1),
                                    )

                if repeat == 1:
                    stream_body()
                else:
                    with tc.For_i(0, repeat, 1) as _i:
                        stream_body(_i)

                # ---------------- POST ----------------
                ssum = cpool.tile([H, BL], F32)
                nc.vector.tensor_reduce(ssum[:], sums[:],
                                        axis=mybir.AxisListType.X,
                                        op=mybir.AluOpType.add)
                sinv = cpool.tile([H, BL], F32)
                nc.vector.reciprocal(sinv[:], ssum[:])
                zn = cpool.tile([H, BL, C], F32)
                for b in range(BL):
                    nc.vector.tensor_scalar_mul(zn[:, b, :], z_ps[b][:],
                                                sinv[:, b:b + 1])

            with (
                tc.tile_pool(name="post", bufs=1) as post,
                tc.tile_pool(name="post_ps", bufs=1, space="PSUM") as ops,
            ):
                wv_sb = post.tile([128, KK, C], F32)
                wo_sb = post.tile([128, KK, C], F32)
                nc.sync.dma_start(wv_sb[:], wv_d.rearrange("(k p) n -> p k n", p=128))
                nc.sync.dma_start(wo_sb[:], wo_d.rearrange("(k p) n -> p k n", p=128))
                bv_sb = post.tile([BL, C], F32)
                bo_sb = post.tile([BL, C], F32)
                nc.sync.dma_start(bv_sb[:], bv_d[:])
                nc.sync.dma_start(bo_sb[:], bo_d[:])

                # transpose z_norm -> zT (128, KK, BL*H)
                zt_sb = post.tile([128, KK, BL * H], F32)
                zt_ps = ops.tile([128, H], F32)
                for k in range(KK):
                    for b in range(BL):
                        nc.tensor.transpose(zt_ps[:],
                                            zn[:, b, k * 128:(k + 1) * 128],
                                            ident[0:H, 0:H])
                        nc.scalar.activation(zt_sb[:, k, b * H:(b + 1) * H],
                                             zt_ps[:], AFT.Copy)

                # y[b, h*D:+D] = z_norm[32b+h] @ Wv[:, h*D:+D]
                y_ps = ops.tile([BL, C], F32)
                for h in range(H):
                    for k in range(KK):
                        nc.tensor.matmul(
                            y_ps[:, h * D:(h + 1) * D],
                            zt_sb[:, k, h::H],
                            wv_sb[:, k, h * D:(h + 1) * D],
                            start=(k == 0), stop=(k == KK - 1),
                        )
                y_sb = post.tile([BL, C], F32)
                nc.vector.tensor_add(y_sb[:], y_ps[:], bv_sb[:])

                # transpose y -> yT
                yt_sb = post.tile([128, KK, BL], F32)
                yt_ps = ops.tile([128, BL], F32)
                for k in range(KK):
                    nc.tensor.transpose(yt_ps[:], y_sb[:, k * 128:(k + 1) * 128],
                                        ident[0:BL, 0:BL])
                    nc.scalar.activation(yt_sb[:, k, :], yt_ps[:], AFT.Copy)

                # out = y @ Wo + bo
                o_ps = ops.tile([BL, C], F32)
                for k in range(KK):
                    for nh in range(2):
                        nc.tensor.matmul(
                            o_ps[:, nh * 512:(nh + 1) * 512],
                            yt_sb[:, k, :],
                            wo_sb[:, k, nh * 512:(nh + 1) * 512],
                            start=(k == 0), stop=(k == KK - 1),
                        )
                o_sb = post.tile([BL, C], F32)
                nc.vector.tensor_add(o_sb[:], o_ps[:], bo_sb[:])
                nc.sync.dma_start(out_d[:], o_sb[:])

    nc.compile()
    return nc


def _host_pe_table():
    position = np.arange(T, dtype=np.float32)[:, None]
    div_term = np.exp(np.arange(0, C, 2, dtype=np.float32)
                      * np.float32(-math.log(10000.0) / C))
    pe = np.zeros((T, C), dtype=np.float32)
    pe[:, 0::2] = np.sin(position * div_term)
    pe[:, 1::2] = np.cos(position * div_term)
    return pe


_NC_CACHE = {}


def kernel(x, Wq, bq, Wkv, bkv, Wo, bo, repeat=1):
    x = np.ascontiguousarray(np.asarray(x, dtype=np.float32))
    Wq = np.asarray(Wq, dtype=np.float32)
    Wkv = np.asarray(Wkv, dtype=np.float32)
    Wo = np.asarray(Wo, dtype=np.float32)
    bq = np.asarray(bq, dtype=np.float32)
    bkv = np.asarray(bkv, dtype=np.float32)
    bo = np.asarray(bo, dtype=np.float32)

    pe = _host_pe_table()
    WkT = np.ascontiguousarray(Wkv[:, :C].T)
    Wv = np.ascontiguousarray(Wkv[:, C:])
    bq2 = np.broadcast_to(bq, (BL, C)).copy()
    bv2 = np.broadcast_to(bkv[C:], (BL, C)).copy()
    bo2 = np.broadcast_to(bo, (BL, C)).copy()
    pe0T = np.broadcast_to(pe[0][:, None], (C, BL)).copy()

    if repeat not in _NC_CACHE:
        _NC_CACHE[repeat] = build_nc(repeat)
    nc = _NC_CACHE[repeat]

    in_maps = []
    for c in range(NCORES):
        xs = x[c * BL:(c + 1) * BL]
        in_maps.append({
            "x": xs,
            "pe": pe,
            "x0T": np.ascontiguousarray(xs[:, 0, :].T),
            "pe0T": pe0T,
            "Wq": Wq, "WkT": WkT, "Wv": Wv, "Wo": Wo,
            "bq2": bq2, "bv2": bv2, "bo2": bo2,
        })
    res = run_bass_kernel_spmd(nc, in_maps, core_ids=list(range(NCORES)),
                               trace=False)
    out = np.concatenate([res.results[c]["out"] for c in range(NCORES)], axis=0)
    return out



# revision 3
# speedup vs baseline: 1.0845x; 1.0845x over previous
"""Trainium2 Bass kernel for decode-style BERT MH self-attention.

Reference computes (B=16, T=8192, C=1024, H=16, D=64):
    x_pe = x + sinusoidal_pe(T, C)
    q  = x_pe[:, :1, :] @ Wq + bq                  (single-query decode)
    kv = x_pe @ Wkv + bkv ; k, v = split
    y  = softmax(q k^T / sqrt(D)) v   -> merge heads -> y @ Wo + bo

Because there is a single query per (b, h), the full K/V projections
(550 GFLOP) collapse algebraically:
    scores[b,h,t] = (Wk_h q_bh) . x_pe[b,t]  + const(b,h)   [const dropped:
                                                             softmax shift-inv]
    y[b,h]        = (attn_b,h . x_pe[b]) @ Wv_h + bv_h      [sum(attn)=1]
so the kernel is one streaming pass over x, memory-bound.

v2 vs v1: pe streamed as bf16 (halves pe DMA), the whole xp pipeline is
bf16 (transposes at 1.0 cyc/row instead of 1.5, PSUM->SBUF copies at 2x
DVE mode, z matmul with 1024-wide bf16 moving), copies rebalanced across
scalar/vector engines.

Sharding: batch B=16 -> 2 per NeuronCore across 8 cores (data parallel,
no collectives).
"""
import math
import sys

sys.path.insert(0, "/opt/trn_rl_repo")

import numpy as np
import ml_dtypes

import concourse.bass as bass
import concourse.mybir as mybir
import concourse.tile as tile
from concourse import bacc
from concourse.bass_utils import run_bass_kernel_spmd
from concourse.masks import make_identity

F32 = mybir.dt.float32
F32R = mybir.dt.float32r
BF16 = mybir.dt.bfloat16

B, T, C, H, D = 16, 8192, 1024, 16, 64
NCORES = 8
BL = B // NCORES          # batches per core = 2
TCH = 512                 # t-chunk (streaming granularity)
NCH = T // TCH            # 16 chunks
G = TCH // 128            # 4 sub-tiles of 128 t per chunk
KK = C // 128             # 8 contraction tiles over channels
AFT = mybir.ActivationFunctionType


def build_nc(repeat: int = 1):
    nc = bacc.Bacc("TRN2", target_bir_lowering=False, debug=False,
                   num_devices=NCORES)

    x_d = nc.dram_tensor("x", [BL, T, C], F32, kind="ExternalInput").ap()
    pe_d = nc.dram_tensor("pe", [T, C], BF16, kind="ExternalInput").ap()
    x0t_d = nc.dram_tensor("x0T", [C, BL], F32, kind="ExternalInput").ap()
    pe0t_d = nc.dram_tensor("pe0T", [C, BL], F32, kind="ExternalInput").ap()
    wq_d = nc.dram_tensor("Wq", [C, C], F32, kind="ExternalInput").ap()
    wkt_d = nc.dram_tensor("WkT", [C, C], F32, kind="ExternalInput").ap()
    wv_d = nc.dram_tensor("Wv", [C, C], F32, kind="ExternalInput").ap()
    wo_d = nc.dram_tensor("Wo", [C, C], F32, kind="ExternalInput").ap()
    bq_d = nc.dram_tensor("bq2", [BL, C], F32, kind="ExternalInput").ap()
    bv_d = nc.dram_tensor("bv2", [BL, C], F32, kind="ExternalInput").ap()
    bo_d = nc.dram_tensor("bo2", [BL, C], F32, kind="ExternalInput").ap()
    out_d = nc.dram_tensor("out", [BL, C], F32, kind="ExternalOutput").ap()

    with tile.TileContext(nc) as tc:
        with tc.tile_pool(name="const", bufs=1) as cpool:
            ident = cpool.tile([128, 128], F32)
            make_identity(nc, ident[:])
            ident_b = cpool.tile([128, 128], BF16)
            nc.vector.tensor_copy(ident_b[:], ident[:])

            # ---------------- PRE: q and qk^T ----------------
            prew = tc.alloc_tile_pool(name="prew", bufs=1)
            wq_sb = prew.tile([128, KK, C], F32)
            wkt_sb = prew.tile([128, KK, C], F32)
            nc.sync.dma_start(wq_sb[:], wq_d.rearrange("(k p) n -> p k n", p=128))
            nc.sync.dma_start(wkt_sb[:], wkt_d.rearrange("(k p) n -> p k n", p=128))

            x0t_sb = cpool.tile([128, KK, BL], F32)
            pe0t_sb = cpool.tile([128, KK, BL], F32)
            nc.sync.dma_start(x0t_sb[:], x0t_d.rearrange("(k p) b -> p k b", p=128))
            nc.sync.dma_start(pe0t_sb[:], pe0t_d.rearrange("(k p) b -> p k b", p=128))
            xp0 = cpool.tile([128, KK, BL], F32)
            nc.vector.tensor_add(xp0[:], x0t_sb[:], pe0t_sb[:])

            bq_sb = cpool.tile([BL, C], F32)
            nc.sync.dma_start(bq_sb[:], bq_d[:])

            with tc.tile_pool(name="pre_ps", bufs=1, space="PSUM") as pps:
                # q = x_pe0 @ Wq + bq  -> (BL, C)
                q_ps = pps.tile([BL, C], F32)
                for k in range(KK):
                    for nh in range(2):
                        nc.tensor.matmul(
                            q_ps[:, nh * 512:(nh + 1) * 512],
                            xp0[:, k, :],
                            wq_sb[:, k, nh * 512:(nh + 1) * 512],
                            start=(k == 0), stop=(k == KK - 1),
                        )
                q_sb = cpool.tile([BL, C], F32)
                nc.vector.tensor_add(q_sb[:], q_ps[:], bq_sb[:])

                # transpose q -> qT (C, BL) as (128, KK, BL)
                qt_ps = pps.tile([128, 128], F32)
                qt_sb = cpool.tile([128, KK, BL], F32)
                for k in range(KK):
                    nc.tensor.transpose(qt_ps[:, 0:BL], q_sb[:, k * 128:(k + 1) * 128],
                                        ident[0:BL, 0:BL])
                    nc.scalar.activation(qt_sb[:, k, :], qt_ps[:, 0:BL], AFT.Copy)

                # qkT[i, b*H+h] = sum_d WkT[h*D+d, i] * qT[h*D+d, b], scaled
                qk_ps = pps.tile([128, KK, BL * H], F32)
                for h in range(H):
                    pp = (h % 2) * 64
                    kj = h // 2
                    for m in range(KK):
                        nc.tensor.matmul(
                            qk_ps[:, m, h::H],
                            wkt_sb[pp:pp + 64, kj, m * 128:(m + 1) * 128],
                            qt_sb[pp:pp + 64, kj, :],
                            start=True, stop=True,
                        )
                qk_sb = cpool.tile([128, KK, BL * H], BF16)
                # fold in the 1/sqrt(D) attention scale
                nc.scalar.activation(qk_sb[:], qk_ps[:], AFT.Copy,
                                     scale=1.0 / math.sqrt(D))
            prew.release()

            # ---------------- STREAM over t ----------------
            sums = cpool.tile([H, BL, NCH], F32)
            nc.gpsimd.memset(sums[:], 1.0)

            with (
                tc.tile_pool(name="xin", bufs=2) as xin_pool,
                tc.tile_pool(name="pein", bufs=2) as pe_pool,
                tc.tile_pool(name="xp", bufs=1) as xp_pool,
                tc.tile_pool(name="xpt", bufs=1) as xpt_pool,
                tc.tile_pool(name="sc_sb", bufs=2) as sc_pool,
                tc.tile_pool(name="at_sb", bufs=2) as at_pool,
                tc.tile_pool(name="tp_ps", bufs=2, space="PSUM") as tp_ps,
                tc.tile_pool(name="sc_ps", bufs=1, space="PSUM") as sc_ps,
                tc.tile_pool(name="at_ps", bufs=1, space="PSUM") as at_ps,
                tc.tile_pool(name="z_ps", bufs=1, space="PSUM") as z_ps_pool,
            ):
                z_ps = [z_ps_pool.tile([H, C], F32, tag=f"z{b}",
                                       name=f"z_ps{b}")
                        for b in range(BL)]

                def stream_body(_iv=None):
                    for tau in range(NCH):
                        pe_t = pe_pool.tile([128, G, C], BF16)
                        nc.sync.dma_start(
                            pe_t[:],
                            pe_d[tau * TCH:(tau + 1) * TCH, :]
                            .rearrange("(g p) i -> p g i", p=128),
                        )
                        for b in range(BL):
                            x_t = xin_pool.tile([128, G, C], F32)
                            nc.sync.dma_start(
                                x_t[:],
                                x_d[b, tau * TCH:(tau + 1) * TCH, :]
                                .rearrange("(g p) i -> p g i", p=128),
                            )
                            xp = xp_pool.tile([128, G, C], BF16, tag=f"xp{b}")
                            nc.vector.tensor_add(xp[:], x_t[:], pe_t[:])

                            # transpose xp -> xpT (i-part, t-free), bf16
                            xpt = xpt_pool.tile([128, KK, TCH], BF16,
                                                tag=f"xpt{b}")
                            for k in range(KK):
                                tp = tp_ps.tile([128, TCH], BF16)
                                for g in range(G):
                                    nc.tensor.transpose(
                                        tp[:, g * 128:(g + 1) * 128],
                                        xp[:, g, k * 128:(k + 1) * 128],
                                        ident_b[:],
                                    )
                                if k % 2 == 0:
                                    nc.scalar.activation(xpt[:, k, :], tp[:],
                                                         AFT.Copy)
                                else:
                                    nc.vector.tensor_copy(xpt[:, k, :], tp[:])

                            # scoresT (h-part, t-free), accumulate over k
                            sc = sc_ps.tile([H, TCH], F32)
                            for k in range(KK):
                                nc.tensor.matmul(
                                    sc[:],
                                    qk_sb[:, k, b * H:(b + 1) * H],
                                    xpt[:, k, :],
                                    start=(k == 0), stop=(k == KK - 1),
                                )
                            # exp (no max subtraction; scores are O(10))
                            at_t = sc_pool.tile([H, TCH], BF16, tag=f"att{b}")
                            nc.scalar.activation(
                                at_t[:], sc[:], AFT.Exp,
                                accum_out=sums[:, b, tau:tau + 1],
                            )
                            # transpose attn chunks to (t-part, h-free)
                            attn = at_pool.tile([128, G, H], BF16, tag=f"attn{b}")
                            ap_ps = at_ps.tile([128, G, H], BF16)
                            for g in range(G):
                                nc.tensor.transpose(
                                    ap_ps[:, g, :],
                                    at_t[:, g * 128:(g + 1) * 128],
                                    ident_b[0:H, 0:H],
                                )
                            nc.scalar.activation(attn[:], ap_ps[:], AFT.Copy)
                            # z += attn^T @ xp   (PSUM accumulate across chunks)
                            for g in range(G):
                                for nh in range(2):
                                    nc.tensor.matmul(
                                        z_ps[b][:, nh * 512:(nh + 1) * 512],
                                        attn[:, g, :],
                                        xp[:, g, nh * 512:(nh + 1) * 512],
                                        start=(tau == 0 and g == 0),
                                        stop=(tau == NCH - 1 and g == G - 1),
                                    )

                if repeat == 1:
                    stream_body()
                else:
                    with tc.For_i(0, repeat, 1) as _i:
                        stream_body(_i)

                # ---------------- POST ----------------
                ssum = cpool.tile([H, BL], F32)
                nc.vector.tensor_reduce(ssum[:], sums[:],
                                        axis=mybir.AxisListType.X,
                                        op=mybir.AluOpType.add)
                sinv = cpool.tile([H, BL], F32)
                nc.vector.reciprocal(sinv[:], ssum[:])
                zn = cpool.tile([H, BL, C], F32)
                for b in range(BL):
                    nc.vector.tensor_scalar_mul(zn[:, b, :], z_ps[b][:],
                                                sinv[:, b:b + 1])

            with (
                tc.tile_pool(name="post", bufs=1) as post,
                tc.tile_pool(name="post_ps", bufs=1, space="PSUM") as ops,
            ):
                wv_sb = post.tile([128, KK, C], F32)
                wo_sb = post.tile([128, KK, C], F32)
                nc.sync.dma_start(wv_sb[:], wv_d.rearrange("(k p) n -> p k n", p=128))
                nc.sync.dma_start(wo_sb[:], wo_d.rearrange("(k p) n -> p k n", p=128))
                bv_sb = post.tile([BL, C], F32)
                bo_sb = post.tile([BL, C], F32)
                nc.sync.dma_start(bv_sb[:], bv_d[:])
                nc.sync.dma_start(bo_sb[:], bo_d[:])

                # transpose z_norm -> zT (128, KK, BL*H)
                zt_sb = post.tile([128, KK, BL * H], F32)
                zt_ps = ops.tile([128, H], F32)
                for k in range(KK):
                    for b in range(BL):
                        nc.tensor.transpose(zt_ps[:],
                                            zn[:, b, k * 128:(k + 1) * 128],
                                            ident[0:H, 0:H])
                        nc.scalar.activation(zt_sb[:, k, b * H:(b + 1) * H],
                                             zt_ps[:], AFT.Copy)

                # y[b, h*D:+D] = z_norm[32b+h] @ Wv[:, h*D:+D]
                y_ps = ops.tile([BL, C], F32)
                for h in range(H):
                    for k in range(KK):
                        nc.tensor.matmul(
                            y_ps[:, h * D:(h + 1) * D],
                            zt_sb[:, k, h::H],
                            wv_sb[:, k, h * D:(h + 1) * D],
                            start=(k == 0), stop=(k == KK - 1),
                        )
                y_sb = post.tile([BL, C], F32)
                nc.vector.tensor_add(y_sb[:], y_ps[:], bv_sb[:])

                # transpose y -> yT
                yt_sb = post.tile([128, KK, BL], F32)
                yt_ps = ops.tile([128, BL], F32)
                for k in range(KK):
                    nc.tensor.transpose(yt_ps[:], y_sb[:, k * 128:(k + 1) * 128],
                                        ident[0:BL, 0:BL])
                    nc.scalar.activation(yt_sb[:, k, :], yt_ps[:], AFT.Copy)

                # out = y @ Wo + bo
                o_ps = ops.tile([BL, C], F32)
                for k in range(KK):
                    for nh in range(2):
                        nc.tensor.matmul(
                            o_ps[:, nh * 512:(nh + 1) * 512],
                            yt_sb[:, k, :],
                            wo_sb[:, k, nh * 512:(nh + 1) * 512],
                            start=(k == 0), stop=(k == KK - 1),
                        )
                o_sb = post.tile([BL, C], F32)
                nc.vector.tensor_add(o_sb[:], o_ps[:], bo_sb[:])
                nc.sync.dma_start(out_d[:], o_sb[:])

    nc.compile()
    return nc


def _host_pe_table():
    position = np.arange(T, dtype=np.float32)[:, None]
    div_term = np.exp(np.arange(0, C, 2, dtype=np.float32)
                      * np.float32(-math.log(10000.0) / C))
    pe = np.zeros((T, C), dtype=np.float32)
    pe[:, 0::2] = np.sin(position * div_term)
    pe[:, 1::2] = np.cos(position * div_term)
    return pe


_NC_CACHE = {}


def kernel(x, Wq, bq, Wkv, bkv, Wo, bo, repeat=1):
    x = np.ascontiguousarray(np.asarray(x, dtype=np.float32))
    Wq = np.asarray(Wq, dtype=np.float32)
    Wkv = np.asarray(Wkv, dtype=np.float32)
    Wo = np.asarray(Wo, dtype=np.float32)
    bq = np.asarray(bq, dtype=np.float32)
    bkv = np.asarray(bkv, dtype=np.float32)
    bo = np.asarray(bo, dtype=np.float32)

    pe = _host_pe_table()
    pe_bf = pe.astype(ml_dtypes.bfloat16)
    WkT = np.ascontiguousarray(Wkv[:, :C].T)
    Wv = np.ascontiguousarray(Wkv[:, C:])
    bq2 = np.broadcast_to(bq, (BL, C)).copy()
    bv2 = np.broadcast_to(bkv[C:], (BL, C)).copy()
    bo2 = np.broadcast_to(bo, (BL, C)).copy()
    pe0T = np.broadcast_to(pe[0][:, None], (C, BL)).copy()

    if repeat not in _NC_CACHE:
        _NC_CACHE[repeat] = build_nc(repeat)
    nc = _NC_CACHE[repeat]

    in_maps = []
    for c in range(NCORES):
        xs = x[c * BL:(c + 1) * BL]
        in_maps.append({
            "x": xs,
            "pe": pe_bf,
            "x0T": np.ascontiguousarray(xs[:, 0, :].T),
            "pe0T": pe0T,
            "Wq": Wq, "WkT": WkT, "Wv": Wv, "Wo": Wo,
            "bq2": bq2, "bv2": bv2, "bo2": bo2,
        })
    res = run_bass_kernel_spmd(nc, in_maps, core_ids=list(range(NCORES)),
                               trace=False)
    out = np.concatenate([res.results[c]["out"] for c in range(NCORES)], axis=0)
    return out


# revision 10
# speedup vs baseline: 2.8124x; 2.5933x over previous
"""Trainium2 Bass kernel for decode-style BERT MH self-attention.

Reference computes (B=16, T=8192, C=1024, H=16, D=64):
    x_pe = x + sinusoidal_pe(T, C)
    q  = x_pe[:, :1, :] @ Wq + bq                  (single-query decode)
    kv = x_pe @ Wkv + bkv ; k, v = split
    y  = softmax(q k^T / sqrt(D)) v   -> merge heads -> y @ Wo + bo

Because there is a single query per (b, h), the full K/V projections
(550 GFLOP) collapse algebraically:
    scores[b,h,t] = (Wk_h q_bh) . x_pe[b,t]  + const(b,h)   [const dropped:
                                                             softmax shift-inv]
    y[b,h]        = (attn_b,h . x_pe[b]) @ Wv_h + bv_h      [sum(attn)=1]
so the kernel is one streaming pass over x, memory-bound.

v4: pe + all weights streamed as bf16; the whole xp pipeline is bf16;
all pools coexist in SBUF (no region reuse) so pre/post weight DMAs and
projections overlap the stream instead of serializing with it; per-batch
adds issue before the PE pipeline so x input buffers recycle fast;
transpose/scores k-loop software-pipelined.

Sharding: batch B=16 -> 2 per NeuronCore across 8 cores (data parallel,
no collectives).
"""
import math
import sys

sys.path.insert(0, "/opt/trn_rl_repo")

import numpy as np
import ml_dtypes

import concourse.bass as bass
import concourse.mybir as mybir
import concourse.tile as tile
from concourse import bacc
from concourse.bass_utils import run_bass_kernel_spmd
from concourse.masks import make_identity

F32 = mybir.dt.float32
F32R = mybir.dt.float32r
BF16 = mybir.dt.bfloat16

B, T, C, H, D = 16, 8192, 1024, 16, 64
NCORES = 8
BL = B // NCORES          # batches per core = 2
TCH = 512                 # t-chunk (streaming granularity)
NCH = T // TCH            # 16 chunks
G = TCH // 128            # 4 sub-tiles of 128 t per chunk
KK = C // 128             # 8 contraction tiles over channels
AFT = mybir.ActivationFunctionType


def build_nc(repeat: int = 1):
    nc = bacc.Bacc("TRN2", target_bir_lowering=False, debug=False,
                   num_devices=NCORES)

    x_d = nc.dram_tensor("x", [BL, T, C], F32, kind="ExternalInput").ap()
    pe_d = nc.dram_tensor("pe", [T, C], BF16, kind="ExternalInput").ap()
    x0t_d = nc.dram_tensor("x0T", [C, BL], F32, kind="ExternalInput").ap()
    pe0t_d = nc.dram_tensor("pe0T", [C, BL], F32, kind="ExternalInput").ap()
    wq_d = nc.dram_tensor("Wq", [C, C], BF16, kind="ExternalInput").ap()
    wkt_d = nc.dram_tensor("WkT", [C, C], BF16, kind="ExternalInput").ap()
    wv_d = nc.dram_tensor("Wv", [C, C], BF16, kind="ExternalInput").ap()
    wo_d = nc.dram_tensor("Wo", [C, C], BF16, kind="ExternalInput").ap()
    bq_d = nc.dram_tensor("bq2", [BL, C], BF16, kind="ExternalInput").ap()
    bv_d = nc.dram_tensor("bv2", [BL, C], BF16, kind="ExternalInput").ap()
    bo_d = nc.dram_tensor("bo2", [BL, C], BF16, kind="ExternalInput").ap()
    out_d = nc.dram_tensor("out", [BL, C], F32, kind="ExternalOutput").ap()

    with tile.TileContext(nc) as tc:
        with (
            tc.tile_pool(name="const", bufs=1) as cpool,
            tc.tile_pool(name="wts", bufs=1) as wpool,
            tc.tile_pool(name="xin", bufs=2) as xin_pool,
            tc.tile_pool(name="pein", bufs=2) as pe_pool,
            tc.tile_pool(name="xp", bufs=2) as xp_pool,
            tc.tile_pool(name="xpt", bufs=2) as xpt_pool,
            tc.tile_pool(name="sc_sb", bufs=2) as sc_pool,
            tc.tile_pool(name="at_sb", bufs=2) as at_pool,
        ):
            ident = cpool.tile([128, 128], F32)
            make_identity(nc, ident[:])
            ident_b = cpool.tile([128, 128], BF16)
            nc.vector.tensor_copy(ident_b[:], ident[:])

            # pre weights (released after qk; wv/wo reuse the space and
            # stream in during the main loop)
            prew = tc.alloc_tile_pool(name="prew", bufs=1)
            wq_sb = prew.tile([128, KK, C], BF16)
            wkt_sb = prew.tile([128, KK, C], BF16)
            nc.sync.dma_start(wq_sb[:], wq_d.rearrange("(k p) n -> p k n", p=128))
            nc.sync.dma_start(wkt_sb[:], wkt_d.rearrange("(k p) n -> p k n", p=128))

            x0t_sb = cpool.tile([128, KK, BL], F32)
            pe0t_sb = cpool.tile([128, KK, BL], F32)
            nc.sync.dma_start(x0t_sb[:], x0t_d.rearrange("(k p) b -> p k b", p=128))
            nc.sync.dma_start(pe0t_sb[:], pe0t_d.rearrange("(k p) b -> p k b", p=128))
            xp0 = cpool.tile([128, KK, BL], BF16)
            nc.vector.tensor_add(xp0[:], x0t_sb[:], pe0t_sb[:])

            bq_sb = cpool.tile([BL, C], BF16)
            bv_sb = cpool.tile([BL, C], BF16)
            bo_sb = cpool.tile([BL, C], BF16)
            nc.sync.dma_start(bq_sb[:], bq_d[:])
            nc.sync.dma_start(bv_sb[:], bv_d[:])
            nc.sync.dma_start(bo_sb[:], bo_d[:])

            qk_sb = cpool.tile([128, KK, BL * H], BF16)
            with tc.tile_pool(name="pre_ps", bufs=1, space="PSUM") as pps:
                # q = x_pe0 @ Wq + bq  -> (BL, C)
                q_ps = pps.tile([BL, C], F32)
                for k in range(KK):
                    for nh in range(2):
                        nc.tensor.matmul(
                            q_ps[:, nh * 512:(nh + 1) * 512],
                            xp0[:, k, :],
                            wq_sb[:, k, nh * 512:(nh + 1) * 512],
                            start=(k == 0), stop=(k == KK - 1),
                        )
                q_sb = cpool.tile([BL, C], BF16)
                nc.vector.tensor_add(q_sb[:], q_ps[:], bq_sb[:])

                # transpose q -> qT (C, BL) as (128, KK, BL)
                qt_ps = pps.tile([128, 128], BF16)
                qt_sb = cpool.tile([128, KK, BL], BF16)
                for k in range(KK):
                    nc.tensor.transpose(qt_ps[:, 0:BL], q_sb[:, k * 128:(k + 1) * 128],
                                        ident_b[0:BL, 0:BL])
                    nc.scalar.activation(qt_sb[:, k, :], qt_ps[:, 0:BL], AFT.Copy)

                # qkT[i, b*H+h] = sum_d WkT[h*D+d, i] * qT[h*D+d, b], scaled
                qk_ps = pps.tile([128, KK, BL * H], F32)
                for h in range(H):
                    pp = (h % 2) * 64
                    kj = h // 2
                    for m in range(KK):
                        nc.tensor.matmul(
                            qk_ps[:, m, h::H],
                            wkt_sb[pp:pp + 64, kj, m * 128:(m + 1) * 128],
                            qt_sb[pp:pp + 64, kj, :],
                            start=True, stop=True,
                        )
                # fold in the 1/sqrt(D) attention scale
                nc.scalar.activation(qk_sb[:], qk_ps[:], AFT.Copy,
                                     scale=1.0 / math.sqrt(D))

            prew.release()
            wv_sb = wpool.tile([128, KK, C], BF16)
            wo_sb = wpool.tile([128, KK, C], BF16)
            nc.sync.dma_start(wv_sb[:], wv_d.rearrange("(k p) n -> p k n", p=128))
            nc.sync.dma_start(wo_sb[:], wo_d.rearrange("(k p) n -> p k n", p=128))

            # ---------------- STREAM over t ----------------
            sums = cpool.tile([H, BL, NCH], F32)
            nc.gpsimd.memset(sums[:], 1.0)

            with (
                tc.tile_pool(name="tp_ps", bufs=2, space="PSUM") as tp_ps,
                tc.tile_pool(name="sc_ps", bufs=1, space="PSUM") as sc_ps,
                tc.tile_pool(name="at_ps", bufs=1, space="PSUM") as at_ps,
                tc.tile_pool(name="z_ps", bufs=1, space="PSUM") as z_ps_pool,
            ):
                # one PSUM tile for both batches: b0 rows [0:16], b1 [32:48]
                # (base partitions 0/32 both legal) -> only 2 PSUM banks.
                z_tile = z_ps_pool.tile([64, C], F32, name="z_ps")
                z_ps = [z_tile[32 * b:32 * b + H, :] for b in range(BL)]

                def stream_body(_iv=None):
                    for tau in range(NCH):
                        pe_t = pe_pool.tile([128, G, C], BF16)
                        nc.sync.dma_start(
                            pe_t[:],
                            pe_d[tau * TCH:(tau + 1) * TCH, :]
                            .rearrange("(g p) i -> p g i", p=128),
                        )
                        # phase 1: DMA + pe-add for BOTH batches first, so x
                        # input buffers recycle quickly and DMA never starves.
                        xps = []
                        for b in range(BL):
                            x_t = xin_pool.tile([128, G, C], F32)
                            nc.sync.dma_start(
                                x_t[:],
                                x_d[b, tau * TCH:(tau + 1) * TCH, :]
                                .rearrange("(g p) i -> p g i", p=128),
                            )
                            xp = xp_pool.tile([128, G, C], BF16, tag=f"xp{b}")
                            # halves: transposes on half 0 start while half 1 adds
                            nc.vector.tensor_add(xp[:, 0:2, :], x_t[:, 0:2, :],
                                                 pe_t[:, 0:2, :])
                            nc.vector.tensor_add(xp[:, 2:4, :], x_t[:, 2:4, :],
                                                 pe_t[:, 2:4, :])
                            xps.append(xp)

                        # phase 2: per-batch transpose/scores/softmax/z
                        for b in range(BL):
                            xp = xps[b]

                            # transpose xp -> xpT (i-part, t-free), bf16.
                            # Software-pipelined with the scores matmuls so PE
                            # always has transpose work while copies drain:
                            #   T(pair0) c0 | T(pair1) S(k0 k1) c1 | ...
                            xpt = xpt_pool.tile([128, KK, TCH], BF16,
                                                tag=f"xpt{b}")
                            sc = sc_ps.tile([H, TCH], F32, tag=f"sc{b}")

                            def transpose_pair(kp):
                                tp = tp_ps.tile([128, 2, TCH], BF16)
                                for kk in range(2):
                                    k = kp * 2 + kk
                                    for g in range(G):
                                        nc.tensor.transpose(
                                            tp[:, kk, g * 128:(g + 1) * 128],
                                            xp[:, g, k * 128:(k + 1) * 128],
                                            ident_b[:],
                                        )
                                # one merged PSUM->SBUF copy per k-pair
                                if kp % 2 == 0:
                                    nc.scalar.activation(
                                        xpt[:, kp * 2:kp * 2 + 2, :], tp[:],
                                        AFT.Copy)
                                else:
                                    nc.vector.tensor_copy(
                                        xpt[:, kp * 2:kp * 2 + 2, :], tp[:])

                            def scores_pair(kp):
                                for kk in range(2):
                                    k = kp * 2 + kk
                                    nc.tensor.matmul(
                                        sc[:],
                                        qk_sb[:, k, b * H:(b + 1) * H],
                                        xpt[:, k, :],
                                        start=(k == 0), stop=(k == KK - 1),
                                    )

                            transpose_pair(0)
                            for kp in range(1, 4):
                                transpose_pair(kp)
                                scores_pair(kp - 1)
                            scores_pair(3)

                            # exp (no max subtraction; scores are O(10))
                            at_t = sc_pool.tile([H, TCH], BF16, tag=f"att{b}")
                            nc.scalar.activation(
                                at_t[:], sc[:], AFT.Exp,
                                accum_out=sums[:, b, tau:tau + 1],
                            )
                            # transpose attn chunks to (t-part, h-free)
                            attn = at_pool.tile([128, G, H], BF16, tag=f"attn{b}")
                            ap_ps = at_ps.tile([128, G, H], BF16)
                            for g in range(G):
                                nc.tensor.transpose(
                                    ap_ps[:, g, :],
                                    at_t[:, g * 128:(g + 1) * 128],
                                    ident_b[0:H, 0:H],
                                )
                            nc.scalar.activation(attn[:], ap_ps[:], AFT.Copy)
                            # z += attn^T @ xp   (PSUM accumulate across chunks)
                            for g in range(G):
                                for nh in range(2):
                                    nc.tensor.matmul(
                                        z_ps[b][:, nh * 512:(nh + 1) * 512],
                                        attn[:, g, :],
                                        xp[:, g, nh * 512:(nh + 1) * 512],
                                        start=(tau == 0 and g == 0),
                                        stop=(tau == NCH - 1 and g == G - 1),
                                    )

                if repeat == 1:
                    stream_body()
                else:
                    with tc.For_i(0, repeat, 1) as _i:
                        stream_body(_i)

                # ---------------- POST ----------------
                ssum = cpool.tile([H, BL], F32)
                nc.vector.tensor_reduce(ssum[:], sums[:],
                                        axis=mybir.AxisListType.X,
                                        op=mybir.AluOpType.add)
                sinv = cpool.tile([H, BL], F32)
                nc.vector.reciprocal(sinv[:], ssum[:])
                zn = cpool.tile([H, BL, C], BF16)
                for b in range(BL):
                    nc.vector.tensor_scalar_mul(zn[:, b, :], z_ps[b][:],
                                                sinv[:, b:b + 1])

            with tc.tile_pool(name="post_ps", bufs=1, space="PSUM") as ops:
                # transpose z_norm -> zT (128, KK, BL*H)
                zt_sb = cpool.tile([128, KK, BL * H], BF16)
                zt_ps = ops.tile([128, H], BF16)
                for k in range(KK):
                    for b in range(BL):
                        nc.tensor.transpose(zt_ps[:],
                                            zn[:, b, k * 128:(k + 1) * 128],
                                            ident_b[0:H, 0:H])
                        nc.scalar.activation(zt_sb[:, k, b * H:(b + 1) * H],
                                             zt_ps[:], AFT.Copy)

                # y[b, h*D:+D] = z_norm[32b+h] @ Wv[:, h*D:+D]
                y_ps = ops.tile([BL, C], F32)
                for h in range(H):
                    for k in range(KK):
                        nc.tensor.matmul(
                            y_ps[:, h * D:(h + 1) * D],
                            zt_sb[:, k, h::H],
                            wv_sb[:, k, h * D:(h + 1) * D],
                            start=(k == 0), stop=(k == KK - 1),
                        )
                y_sb = cpool.tile([BL, C], BF16)
                nc.vector.tensor_add(y_sb[:], y_ps[:], bv_sb[:])

                # transpose y -> yT
                yt_sb = cpool.tile([128, KK, BL], BF16)
                yt_ps = ops.tile([128, BL], BF16)
                for k in range(KK):
                    nc.tensor.transpose(yt_ps[:], y_sb[:, k * 128:(k + 1) * 128],
                                        ident_b[0:BL, 0:BL])
                    nc.scalar.activation(yt_sb[:, k, :], yt_ps[:], AFT.Copy)

                # out = y @ Wo + bo
                o_ps = ops.tile([BL, C], F32)
                for k in range(KK):
                    for nh in range(2):
                        nc.tensor.matmul(
                            o_ps[:, nh * 512:(nh + 1) * 512],
                            yt_sb[:, k, :],
                            wo_sb[:, k, nh * 512:(nh + 1) * 512],
                            start=(k == 0), stop=(k == KK - 1),
                        )
                o_sb = cpool.tile([BL, C], F32)
                nc.vector.tensor_add(o_sb[:], o_ps[:], bo_sb[:])
                nc.sync.dma_start(out_d[:], o_sb[:])

    nc.compile()
    return nc


def _host_pe_table():
    position = np.arange(T, dtype=np.float32)[:, None]
    div_term = np.exp(np.arange(0, C, 2, dtype=np.float32)
                      * np.float32(-math.log(10000.0) / C))
    pe = np.zeros((T, C), dtype=np.float32)
    pe[:, 0::2] = np.sin(position * div_term)
    pe[:, 1::2] = np.cos(position * div_term)
    return pe


_NC_CACHE = {}


def kernel(x, Wq, bq, Wkv, bkv, Wo, bo, repeat=1):
    x = np.ascontiguousarray(np.asarray(x, dtype=np.float32))
    Wq = np.asarray(Wq, dtype=np.float32)
    Wkv = np.asarray(Wkv, dtype=np.float32)
    Wo = np.asarray(Wo, dtype=np.float32)
    bq = np.asarray(bq, dtype=np.float32)
    bkv = np.asarray(bkv, dtype=np.float32)
    bo = np.asarray(bo, dtype=np.float32)

    pe = _host_pe_table()
    bf = ml_dtypes.bfloat16
    pe_bf = pe.astype(bf)
    WkT = np.ascontiguousarray(Wkv[:, :C].T).astype(bf)
    Wv = np.ascontiguousarray(Wkv[:, C:]).astype(bf)
    bq2 = np.broadcast_to(bq, (BL, C)).copy()
    bv2 = np.broadcast_to(bkv[C:], (BL, C)).copy()
    bo2 = np.broadcast_to(bo, (BL, C)).copy()
    pe0T = np.broadcast_to(pe[0][:, None], (C, BL)).copy()

    if repeat not in _NC_CACHE:
        _NC_CACHE[repeat] = build_nc(repeat)
    nc = _NC_CACHE[repeat]

    in_maps = []
    for c in range(NCORES):
        xs = x[c * BL:(c + 1) * BL]
        in_maps.append({
            "x": xs,
            "pe": pe_bf,
            "x0T": np.ascontiguousarray(xs[:, 0, :].T),
            "pe0T": pe0T,
            "Wq": Wq.astype(bf), "WkT": WkT, "Wv": Wv,
            "Wo": Wo.astype(bf),
            "bq2": bq2.astype(bf), "bv2": bv2.astype(bf), "bo2": bo2.astype(bf),
        })
    res = run_bass_kernel_spmd(nc, in_maps, core_ids=list(range(NCORES)),
                               trace=False)
    out = np.concatenate([res.results[c]["out"] for c in range(NCORES)], axis=0)
    return out
